# revision 2
# baseline (speedup 1.0000x reference)
"""Causal MHA (B=4, T=2048, D=1024, H=16) on 8 TRN2 NeuronCores — fp8 edition.

Sharding: core c -> batch b = c // 2, head-group g = c % 2 (8 heads each).
All five matmul families run fp8e4m3 DoubleRow (0.5 PE cycles/row):

  qkv   : x^T pre-packed [128, th, u, i, t] pairing d=256u+128i+p; W same.
  S     : Q,K stored [128, 2, T] fp8 — 4 heads per tile in 32-partition
          strips (strip 32*(m%4)), pair dim = dh//32.  1/sqrt(dh) folded
          into the exp scale, not into Q (fp8 subnormal hazard).
  AV    : probs written by exp as fp8 into [128, kpair, head, 512] tiles;
          V packed [128, kpair, head, 128] (col 64 = ones -> rowsums in
          psum; cols 65:128 zero pad — dual-fp8 ldweights wants full
          tiles).  One DoubleRow matmul contracts TWO k-tiles.
  oproj : onorm fp8 [128, 2, T] pairing head-pairs; W_o fp8 pairs.

exp: off-diagonal tiles alternate ScalarE Exp(scale=1/8)->fp8 and DVE
Schraudolph int8-saturation -> e4m3 bits.  Diagonal tiles split: DVE does
the 128-wide triangular block (mask folded into a f32 bias operand; -1e9
saturates int8 to 0x80 = -0.0), ScalarE the unmasked tail, GpSimd memsets
the dead prefix.  Row-sum reciprocal via DRAM-respread as baseline.
Output partials stored bf16; host sums pairs in f32 and adds b_o.
"""

import sys

sys.path.insert(0, "/opt/trn_rl_repo")

import numpy as np

import concourse.bass as bass
import concourse.mybir as mybir
from concourse.bass_utils import run_bass_kernel_spmd
from concourse.tile import TileContext

F32 = mybir.dt.float32
BF16 = mybir.dt.bfloat16
FP8 = mybir.dt.float8e4
I8 = mybir.dt.int8
I16 = mybir.dt.int16
EXP = mybir.ActivationFunctionType.Exp
COPY = mybir.ActivationFunctionType.Copy
MULT = mybir.AluOpType.mult
ADD = mybir.AluOpType.add
DR = mybir.MatmulPerfMode.DoubleRow

B, T, D, H = 4, 2048, 1024, 16
DH = D // H          # 64
HPC = H // 2         # heads per core = 8
DPC = HPC * DH       # 512 projected dims per core
N_CORES = 8
QC = 512             # q-chunk width

SCH_A8 = 1.4426950408889634   # exp(s/8): (1/8) * 8*log2(e)
SCH_B8 = 56.0 - 0.4625        # 7*8 with mean-centering correction
SCH_MASKED8 = SCH_B8 - 1.0e9  # saturates int8 -> -128 -> e4m3 -0.0
SCH_A16 = 184.66496030 / 8.0  # bf16 Schraudolph with /8 folded in
SCH_B16 = 16256.0 - 7.4
SCH_MASKED16 = SCH_B16 - 1.0e9


def split_excess_waits(nc, cap=1):
    """walrus limits sync-wait slots per ISA instruction (1 for several
    structs).  Move excess waits onto InstEventSemaphore instructions
    inserted just before the offender on the same engine."""
    n_split = 0
    for f in nc.m.functions:
        for blk in f.blocks:
            insts = blk.instructions
            out = []
            changed = False
            for inst in insts:
                si = inst.sync_info
                waits = list(si.on_wait) if si is not None else []
                if len(waits) > cap:
                    for j, w in enumerate(waits[:-cap]):
                        ev = mybir.InstEventSemaphore(
                            name=f"{inst.name}-w{j}", ins=[], outs=[]
                        )
                        ev.engine = inst.engine
                        ev.sync_info = mybir.SyncInfo(on_wait=[w], on_update=[])
                        out.append(ev)
                        n_split += 1
                    inst.sync_info = mybir.SyncInfo(
                        on_wait=waits[-cap:], on_update=list(si.on_update)
                    )
                    changed = True
                out.append(inst)
            if changed:
                blk.instructions = out
    return n_split


INST_LABELS = {}


def build():
    nc = bass.Bass(target_bir_lowering=False)

    _label = ["init"]

    def set_label(s):
        _label[0] = s

    for eng in (nc.tensor, nc.vector, nc.scalar, nc.gpsimd, nc.sync):
        orig = eng.add_instruction

        def wrapped(inst, _orig=orig):
            r = _orig(inst)
            try:
                INST_LABELS[inst.name] = _label[0]
            except Exception:
                pass
            return r

        eng.add_instruction = wrapped

    # x8: [128, th, u, i, tcol]  (d = 256u + 128i + p, t = 1024*th + tcol)
    x8_d = nc.dram_tensor("x8", [128, 2 * 4 * 2 * 1024], FP8, kind="ExternalInput")
    # wqk8: [p, j, u, i, c]  j=0..3 Q (j=2g+iq), j=4..7 K
    wqk8_d = nc.dram_tensor("wqk8", [128, 8 * 4 * 2 * 128], FP8, kind="ExternalInput")
    # wv8: [p, u, i, v]
    wv8_d = nc.dram_tensor("wv8", [128, 4 * 2 * 512], FP8, kind="ExternalInput")
    # wo8: [p, g, i, outcol]
    wo8_d = nc.dram_tensor("wo8", [128, 2 * 2 * 1024], FP8, kind="ExternalInput")
    bqk_d = nc.dram_tensor("bqk", [128, 8], F32, kind="ExternalInput")
    bv_d = nc.dram_tensor("bv", [1, DPC], BF16, kind="ExternalInput")
    # trimask bias: [p, head, 128] f32 (keep: SCH_B8, masked: SCH_B8-1e9)
    bmask_d = nc.dram_tensor("bmask8", [128, 2 * 128], F32, kind="ExternalInput")
    # ---- bf16 epilogue inputs (exact recompute of rows 0..127) ----
    xbf_d = nc.dram_tensor("xbf", [128, 8 * 128], BF16, kind="ExternalInput")
    wqkbf_d = nc.dram_tensor("wqkbf", [128, 8 * 8 * 128], BF16, kind="ExternalInput")
    wvbf_d = nc.dram_tensor("wvbf", [128, 8 * 512], BF16, kind="ExternalInput")
    wobf_d = nc.dram_tensor("wobf", [128, 4 * 1024], BF16, kind="ExternalInput")
    bqke_d = nc.dram_tensor("bqke", [128, 8], F32, kind="ExternalInput")
    bmaske_d = nc.dram_tensor("bmaske", [128, 2 * 128], F32, kind="ExternalInput")
    out_d = nc.dram_tensor("out", [T, D], BF16, kind="ExternalOutput")
    out2_d = nc.dram_tensor("out2", [128, D], BF16, kind="ExternalOutput")
    rsraw_d = nc.dram_tensor("rsraw", [4, 4096], BF16)  # raw rowsums / chunk
    rrec_d = nc.dram_tensor("rrec", [4, 4096], BF16)    # reciprocals / chunk
    rsraw2_d = nc.dram_tensor("rsraw2", [1, 1024], BF16)
    rrec2_d = nc.dram_tensor("rrec2", [1, 1024], BF16)

    with TileContext(nc) as tc:
        with (
            tc.tile_pool(name="const", bufs=1) as constp,
            tc.tile_pool(name="wstream", bufs=8) as wp,
            tc.tile_pool(name="xt", bufs=2) as xtp,
            tc.tile_pool(name="qk", bufs=1) as qkp,
            tc.tile_pool(name="vaug", bufs=1) as vp,
            tc.tile_pool(name="onorm", bufs=1) as onp,
            tc.tile_pool(name="pt", bufs=4) as ptp,
            tc.tile_pool(name="sc", bufs=8) as scp,
            tc.tile_pool(name="rs", bufs=4) as rsp,
            tc.tile_pool(name="bc", bufs=8) as bcp,
            tc.tile_pool(name="osb", bufs=3) as osbp,
            tc.tile_pool(name="ps", bufs=4, space="PSUM") as psp,
            tc.tile_pool(name="spair", bufs=2, space="PSUM") as spp,
        ):
            # qk_sb[j]: bf16 [128, T], heads pair-stacked (64+64) as in the
            # bf16 baseline — S matmuls gain nothing from dual-fp8 (the win
            # is per-row contraction depth, and S only contracts 64), so
            # keep them bf16 with concurrent PE quadrants.
            qk_sb = [
                qkp.tile([128, T], BF16, tag=f"qk{j}", name=f"qk{j}")
                for j in range(8)
            ]
            # vaug2[tp]: [128, 2, 8, 128]  (k = 256*tp+128*i+p, head, cols)
            vaug2 = [
                vp.tile([128, 2, HPC, 2 * DH], FP8, tag=f"v{t}", name=f"v{t}")
                for t in range(8)
            ]
            # onorm8[g]: [128, 2, T]  (odim = 64*(4g+2i+p//64) + p%64)
            onorm8 = [
                onp.tile([128, 2, T], FP8, tag=f"on{g}", name=f"on{g}")
                for g in range(2)
            ]

            set_label("qkv")

            def load_wj(th, j):
                w_j = wp.tile([128, 4, 2, 128], FP8, tag="wqk", name=f"w{th}_{j}")
                nc.sync.dma_start(
                    w_j[:],
                    wqk8_d[:, 1024 * j : 1024 * (j + 1)].rearrange(
                        "p (u i c) -> p u i c", u=4, i=2
                    ),
                )
                return w_j

            xts = {}

            def load_x(th):
                xt = []
                for u in range(4):
                    x_t = xtp.tile(
                        [128, 2, 1024], FP8, tag=f"xt{u}", name=f"xt{th}_{u}"
                    )
                    eng = nc.sync if u % 2 == 0 else nc.gpsimd
                    off = (th * 4 + u) * 2048
                    eng.dma_start(
                        x_t[:],
                        x8_d[:, off : off + 2048].rearrange(
                            "p (i c) -> p i c", i=2
                        ),
                    )
                    xt.append(x_t)
                xts[th] = xt

            # ---- first loads: w(j=0) + x(th0) before any consts ----
            w_first = load_wj(0, 0)
            load_x(0)

            set_label("const")
            wv_sb = constp.tile([128, 4, 2, 512], FP8, tag="wv")
            nc.gpsimd.dma_start(
                wv_sb[:], wv8_d[:].rearrange("p (u i v) -> p u i v", u=4, i=2)
            )
            bqk_sb = constp.tile([128, 8], F32, tag="bqk")
            nc.gpsimd.dma_start(bqk_sb[:], bqk_d[:])
            # V bias broadcast to all partitions (DVE add, no bias matmul)
            bvb_sb = constp.tile([128, 8, DH], BF16, tag="bvb")
            nc.gpsimd.dma_start(
                bvb_sb[:].rearrange("p h d -> p (h d)"),
                bass.AP(bv_d, 0, [[0, 128], [1, DPC]]),
            )
            bmask_sb = constp.tile([128, 2, 128], F32, tag="bmask")
            nc.gpsimd.dma_start(
                bmask_sb[:],
                bmask_d[:].rearrange("p (h q) -> p h q", h=2),
            )
            # wo early: oproj chunk 0 runs during t-half-1 projections
            wo_sb = constp.tile([128, 2, 2, 1024], FP8, tag="wo")
            nc.gpsimd.dma_start(
                wo_sb[:], wo8_d[:].rearrange("p (g i c) -> p g i c", g=2, i=2)
            )
            # zero the pad cols of vaug so psum garbage rows stay finite
            for tp_i in range(8):
                nc.gpsimd.memset(vaug2[tp_i][:, :, :, DH + 1 :], 0.0)

            # bf16 epilogue constants (rows 0..127 exact recompute)
            xbf_sb = constp.tile([128, 8, 128], BF16, tag="xbf")
            nc.gpsimd.dma_start(
                xbf_sb[:], xbf_d[:].rearrange("p (d t) -> p d t", d=8)
            )
            wqkbf_sb = constp.tile([128, 8, 8, 128], BF16, tag="wqkbf")
            nc.gpsimd.dma_start(
                wqkbf_sb[:],
                wqkbf_d[:].rearrange("p (j d c) -> p j d c", j=8, d=8),
            )
            wvbf_sb = constp.tile([128, 8, 512], BF16, tag="wvbf")
            nc.gpsimd.dma_start(
                wvbf_sb[:], wvbf_d[:].rearrange("p (d v) -> p d v", d=8)
            )
            wobf_sb = constp.tile([128, 4, 1024], BF16, tag="wobf")
            nc.gpsimd.dma_start(
                wobf_sb[:], wobf_d[:].rearrange("p (h c) -> p h c", h=4)
            )
            bqke_sb = constp.tile([128, 8], F32, tag="bqke")
            nc.gpsimd.dma_start(bqke_sb[:], bqke_d[:])
            bmaske_sb = constp.tile([128, 2, 128], F32, tag="bmaske")
            nc.gpsimd.dma_start(
                bmaske_sb[:], bmaske_d[:].rearrange("p (h q) -> p h q", h=2)
            )

            # ---- phase 1: projections for one t-half ----
            def proj_half(th):
                t0 = th * (T // 2)
                xt = xts[th]
                set_label("qkv")
                w_tiles = [
                    w_first if (th == 0 and j == 0) else load_wj(th, j)
                    for j in range(8)
                ]
                for j in range(8):
                    w_j = w_tiles[j]
                    ps0 = psp.tile([128, 512], F32, tag="ps", name=f"q{th}{j}a")
                    ps1 = psp.tile([128, 512], F32, tag="ps", name=f"q{th}{j}b")
                    for u in range(4):
                        nc.tensor.matmul(
                            ps0[:],
                            w_j[:, u, :, :],
                            xt[u][:, :, 0:512],
                            start=(u == 0),
                            stop=(u == 3),
                            perf_mode=DR,
                        )
                        nc.tensor.matmul(
                            ps1[:],
                            w_j[:, u, :, :],
                            xt[u][:, :, 512:1024],
                            start=(u == 0),
                            stop=(u == 3),
                            perf_mode=DR,
                        )
                    nc.vector.tensor_scalar_add(
                        qk_sb[j][:, t0 : t0 + 512], ps0[:], bqk_sb[:, j : j + 1]
                    )
                    nc.vector.tensor_scalar_add(
                        qk_sb[j][:, t0 + 512 : t0 + 1024],
                        ps1[:],
                        bqk_sb[:, j : j + 1],
                    )

                # V: bias via DVE broadcast-add; ones col via memset
                set_label("vproj")
                for tt in range(8):
                    tg = th * 8 + tt
                    tp_i, i = tg // 2, tg % 2
                    ps = psp.tile([128, 512], F32, tag="ps", name=f"v{th}{tt}")
                    for u in range(4):
                        nc.tensor.matmul(
                            ps[:],
                            xt[u][:, :, 128 * tt : 128 * (tt + 1)],
                            wv_sb[:, u, :, :],
                            start=(u == 0),
                            stop=(u == 3),
                            perf_mode=DR,
                        )
                    nc.vector.tensor_tensor(
                        vaug2[tp_i][:, i, :, 0:DH],
                        ps[:].rearrange("p (h d) -> p h d", h=HPC),
                        bvb_sb[:],
                        ADD,
                    )
                    nc.gpsimd.memset(vaug2[tp_i][:, i, :, DH : DH + 1], 1.0)

            # ---- phase 2: attention ----
            _off_rr = [0]

            def emit_exp(c, t, qsP, sp, pt, i_p):
                """exp of k-tile t's scores (both heads) into pt[:, i_p].
                The pair covers cols [qsP:512]."""
                j = t - 4 * c
                spv = sp[:].rearrange("p (h q) -> p h q", h=2)
                if j < 0:
                    # fully off-diagonal: whole-tile instructions amortize
                    # the ~350ns per-instruction overhead; rotate ACT:DVE
                    # 5:4 to equalize total engine load
                    eng = "act" if _off_rr[0] % 9 < 5 else "dve"
                    _off_rr[0] += 1
                    ptv = pt[:, i_p, :, :]
                    if eng == "act":
                        nc.scalar.activation(ptv, spv, EXP, scale=0.125)
                    else:
                        nc.vector.tensor_scalar(
                            ptv.bitcast(I8), spv, SCH_A8, SCH_B8, MULT, ADD
                        )
                    return
                qs = 128 * j
                if qs > qsP:
                    # dead prefix of the second diag tile of the pair
                    nc.gpsimd.memset(pt[:, i_p, :, qsP:qs], 0.0)
                # triangular block on DVE (mask folded into f32 bias)
                nc.vector.scalar_tensor_tensor(
                    pt[:, i_p, :, qs : qs + 128].bitcast(I8),
                    spv[:, :, qs : qs + 128],
                    SCH_A8,
                    bmask_sb[:],
                    MULT,
                    ADD,
                )
                # unmasked tail on ACT
                if qs + 128 < 512:
                    nc.scalar.activation(
                        pt[:, i_p, :, qs + 128 : 512],
                        spv[:, :, qs + 128 : 512],
                        EXP,
                        scale=0.125,
                    )

            _chunk_sc = {}

            def attn_block2(c, hpA, hpB):
                """two head-pairs interleaved: while one stream's exp runs,
                the other stream's S/AV matmuls keep every engine fed."""
                set_label("attn")
                q0 = QC * c
                ktiles = 4 * (c + 1)
                oPS = {
                    hp: [
                        psp.tile([128, 512], F32, tag="ps", name=f"oA{c}{hp}"),
                        psp.tile([128, 512], F32, tag="ps", name=f"oB{c}{hp}"),
                    ]
                    for hp in (hpA, hpB)
                }

                def s_mm(hp, t):
                    j = t - 4 * c
                    qs = 128 * j if j >= 0 else 0
                    sp = spp.tile([128, 1024], F32, tag="sp", name=f"sp{hp}_{t}")
                    for half, base in ((0, 0), (1, 64)):
                        nc.tensor.matmul(
                            sp[:, 512 * half + qs : 512 * (half + 1)],
                            qk_sb[4 + hp][base : base + 64, 128 * t : 128 * (t + 1)],
                            qk_sb[hp][base : base + 64, q0 + qs : q0 + QC],
                            start=True,
                            stop=True,
                            tile_position=(base, 0),
                        )
                    return sp

                def av_mm(hp, tp_i, qsP, pt, last):
                    for half in range(2):
                        hh = 2 * hp + half
                        nc.tensor.matmul(
                            oPS[hp][half][:, qsP:512],
                            vaug2[tp_i][:, :, hh, :],
                            pt[:, :, half, qsP:512],
                            start=(tp_i == 0),
                            stop=last,
                            perf_mode=DR,
                            skip_group_check=True,
                        )

                npairs = ktiles // 2
                for tp_i in range(npairs):
                    t0_, t1_ = 2 * tp_i, 2 * tp_i + 1
                    j0 = t0_ - 4 * c
                    qsP = 128 * j0 if j0 >= 0 else 0
                    ptd = {}
                    for hp in (hpA, hpB):
                        sp0 = s_mm(hp, t0_)
                        sp1 = s_mm(hp, t1_)
                        pt = ptp.tile(
                            [128, 2, 2, 512], FP8, tag="pt", name=f"pt{hp}_{t0_}"
                        )
                        emit_exp(c, t0_, qsP, sp0, pt, 0)
                        emit_exp(c, t1_, qsP, sp1, pt, 1)
                        ptd[hp] = pt
                    for hp in (hpA, hpB):
                        av_mm(hp, tp_i, qsP, ptd[hp], last=(tp_i == npairs - 1))

                # evict raw o + rowsum row together (ACT, one [65,512] copy);
                # rowsum row DMAs to DRAM for the chunk-batched respread
                set_label("norm")
                for hp in (hpA, hpB):
                    for half in range(2):
                        idx = 2 * hp + half
                        sc = scp.tile(
                            [DH + 1, 512], BF16, tag="sc", name=f"sc{c}{hp}{half}"
                        )
                        nc.scalar.activation(
                            sc[:], oPS[hp][half][0 : DH + 1, :], COPY
                        )
                        nc.gpsimd.dma_start(
                            rsraw_d[c : c + 1, idx * 512 : (idx + 1) * 512],
                            sc[DH : DH + 1, :],
                        )
                        _chunk_sc[(c, hp, half)] = sc

            def norm_chunk(c):
                """batched reciprocal + normalize for all 4 hp of chunk c"""
                set_label("norm")
                rload = rsp.tile([128, 32], BF16, tag="rload")
                nc.sync.dma_start(
                    rload[:], bass.AP(rsraw_d, c * 4096, [[32, 128], [1, 32]])
                )
                rrec = rsp.tile([128, 32], BF16, tag="rrec")
                with nc.allow_low_precision(
                    reason="bf16 softmax-normalizer reciprocal; well inside "
                    "the output tolerance"
                ):
                    nc.vector.reciprocal(rrec[:], rload[:])
                nc.sync.dma_start(
                    bass.AP(rrec_d, c * 4096, [[32, 128], [1, 32]]), rrec[:]
                )
                for hp in range(4):
                    for half in range(2):
                        m = 2 * hp + half
                        gg, i_o = m // 4, (m % 4) // 2
                        bc = bcp.tile([64, 512], BF16, tag="bc")
                        nc.sync.dma_start(
                            bc[:],
                            bass.AP(
                                rrec_d,
                                c * 4096 + m * 512,
                                [[0, 64], [1, 512]],
                            ),
                        )
                        sc = _chunk_sc.pop((c, hp, half))
                        nc.gpsimd.tensor_tensor(
                            onorm8[gg][
                                64 * (m % 2) : 64 * (m % 2) + 64,
                                i_o,
                                QC * c : QC * (c + 1),
                            ],
                            sc[0:DH, :],
                            bc[:],
                            MULT,
                        )

            def attn_chunk(c):
                attn_block2(c, 0, 1)
                attn_block2(c, 2, 3)
                norm_chunk(c)

            def oproj_chunk(c):
                oproj_qts(4 * c, 4 * c + 4)

            # ---- phase 3: output projection ----
            def oproj_qts(q0_, q1_):
                set_label("oproj")
                for qt in range(q0_, q1_):
                    ps0 = psp.tile([128, 512], F32, tag="ps", name=f"o{qt}a")
                    ps1 = psp.tile([128, 512], F32, tag="ps", name=f"o{qt}b")
                    for gg in range(2):
                        nc.tensor.matmul(
                            ps0[:],
                            onorm8[gg][:, :, 128 * qt : 128 * (qt + 1)],
                            wo_sb[:, gg, :, 0:512],
                            start=(gg == 0),
                            stop=(gg == 1),
                            perf_mode=DR,
                        )
                        nc.tensor.matmul(
                            ps1[:],
                            onorm8[gg][:, :, 128 * qt : 128 * (qt + 1)],
                            wo_sb[:, gg, :, 512:1024],
                            start=(gg == 0),
                            stop=(gg == 1),
                            perf_mode=DR,
                        )
                    for dc, ps in ((0, ps0), (1, ps1)):
                        osb = osbp.tile([128, 512], BF16, tag="osb")
                        nc.scalar.activation(osb[:], ps[:], COPY)
                        nc.gpsimd.dma_start(
                            out_d[
                                128 * qt : 128 * (qt + 1),
                                512 * dc : 512 * (dc + 1),
                            ],
                            osb[:],
                        )

            # ---- bf16 epilogue: exact recompute of output rows 0..127 ----
            def epilogue():
                set_label("epi")
                # Q,K: two psum tiles, 4 j-blocks each (col range 128*(j%4))
                qk_ep = constp.tile([128, 8, 128], BF16, tag="qk_ep")
                for grp in range(2):  # 0 = Q blocks 0..3, 1 = K blocks 4..7
                    ps = psp.tile([128, 512], F32, tag="ps", name=f"eqk{grp}")
                    for jj in range(4):
                        j = 4 * grp + jj
                        for dt in range(8):
                            nc.tensor.matmul(
                                ps[:, 128 * jj : 128 * (jj + 1)],
                                wqkbf_sb[:, j, dt, :],
                                xbf_sb[:, dt, :],
                                start=(dt == 0),
                                stop=(dt == 7),
                                skip_group_check=True,
                            )
                    for jj in range(4):
                        j = 4 * grp + jj
                        nc.vector.tensor_scalar_add(
                            qk_ep[:, j, :],
                            ps[:, 128 * jj : 128 * (jj + 1)],
                            bqke_sb[:, j : j + 1],
                        )
                # V: [128 t, 512 v] + bias + ones col
                vep = constp.tile([128, HPC, DH + 1], BF16, tag="vep")
                psv = psp.tile([128, 512], F32, tag="ps", name="evps")
                for dt in range(8):
                    nc.tensor.matmul(
                        psv[:],
                        xbf_sb[:, dt, :],
                        wvbf_sb[:, dt, :],
                        start=(dt == 0),
                        stop=(dt == 7),
                    )
                nc.vector.tensor_tensor(
                    vep[:, :, 0:DH],
                    psv[:].rearrange("p (h d) -> p h d", h=HPC),
                    bvb_sb[:],
                    ADD,
                )
                nc.gpsimd.memset(vep[:, :, DH : DH + 1], 1.0)
                # attention per head pair (single diagonal k-tile)
                onorm_ep = constp.tile([128, 4, 128], BF16, tag="onorm_ep")
                for hp in range(4):
                    spE = spp.tile([128, 1024], F32, tag="sp", name=f"esp{hp}")
                    for half in range(2):
                        # halves in different psum banks (cols 0 / 512)
                        nc.tensor.matmul(
                            spE[:, 512 * half : 512 * half + 128],
                            qk_ep[64 * half : 64 * half + 64, 4 + hp, :],
                            qk_ep[64 * half : 64 * half + 64, hp, :],
                            start=True,
                            stop=True,
                            tile_position=(64 * half, 0),
                        )
                    ptE = bcp.tile([128, 2, 128], BF16, tag="ptE", name=f"ept{hp}")
                    nc.vector.scalar_tensor_tensor(
                        ptE[:].bitcast(I16),
                        spE[:].rearrange("p (h q) -> p h q", h=2)[:, :, 0:128],
                        SCH_A16,
                        bmaske_sb[:],
                        MULT,
                        ADD,
                    )
                    oE = psp.tile([128, 512], F32, tag="ps", name=f"eo{hp}")
                    for half in range(2):
                        nc.tensor.matmul(
                            oE[0 : DH + 1, 128 * half : 128 * (half + 1)],
                            vep[:, 2 * hp + half, :],
                            ptE[:, half, :],
                            start=True,
                            stop=True,
                            skip_group_check=True,
                        )
                    # rowsums for both halves -> DRAM respread
                    rrowE = rsp.tile([1, 256], BF16, tag="rrowE")
                    nc.vector.tensor_copy(
                        out=rrowE[:], in_=oE[DH : DH + 1, 0:256]
                    )
                    nc.gpsimd.dma_start(
                        rsraw2_d[0:1, 256 * hp : 256 * (hp + 1)], rrowE[:]
                    )
                    scE = scp.tile([64, 256], BF16, tag="scE", name=f"scE{hp}")
                    nc.scalar.activation(scE[:], oE[0:DH, 0:256], COPY)
                    rloadE = rsp.tile([128, 2], BF16, tag="rloadE")
                    nc.sync.dma_start(
                        rloadE[:],
                        bass.AP(rsraw2_d, 256 * hp, [[2, 128], [1, 2]]),
                    )
                    rrecE = rsp.tile([128, 2], BF16, tag="rrecE")
                    with nc.allow_low_precision(
                        reason="bf16 softmax-normalizer reciprocal"
                    ):
                        nc.vector.reciprocal(rrecE[:], rloadE[:])
                    nc.sync.dma_start(
                        bass.AP(rrec2_d, 256 * hp, [[2, 128], [1, 2]]),
                        rrecE[:],
                    )
                    for half in range(2):
                        bcE = bcp.tile([64, 128], BF16, tag="bcE")
                        nc.sync.dma_start(
                            bcE[:],
                            bass.AP(
                                rrec2_d,
                                256 * hp + 128 * half,
                                [[0, 64], [1, 128]],
                            ),
                        )
                        nc.vector.tensor_tensor(
                            onorm_ep[64 * half : 64 * half + 64, hp, :],
                            scE[:, 128 * half : 128 * (half + 1)],
                            bcE[:],
                            MULT,
                        )
                # oproj for the 128 rows
                psa = psp.tile([128, 512], F32, tag="ps", name="eoa")
                psb = psp.tile([128, 512], F32, tag="ps", name="eob")
                for hp in range(4):
                    nc.tensor.matmul(
                        psa[:],
                        onorm_ep[:, hp, :],
                        wobf_sb[:, hp, 0:512],
                        start=(hp == 0),
                        stop=(hp == 3),
                    )
                    nc.tensor.matmul(
                        psb[:],
                        onorm_ep[:, hp, :],
                        wobf_sb[:, hp, 512:1024],
                        start=(hp == 0),
                        stop=(hp == 3),
                    )
                for dc, ps in ((0, psa), (1, psb)):
                    osb = osbp.tile([128, 512], BF16, tag="osb")
                    nc.scalar.activation(osb[:], ps[:], COPY)
                    nc.gpsimd.dma_start(
                        out2_d[:, 512 * dc : 512 * (dc + 1)], osb[:]
                    )

            # ---- emission order ----
            proj_half(0)
            load_x(1)          # prefetch t-half-1 x during early attention
            attn_chunk(0)
            attn_chunk(1)
            oproj_chunk(0)
            proj_half(1)
            oproj_chunk(1)
            attn_chunk(3)      # big chunk first: its oproj hides under a2
            attn_block2(2, 0, 1)
            oproj_qts(12, 14)  # chunk-3 oproj interleaved into chunk 2
            attn_block2(2, 2, 3)
            oproj_qts(14, 16)
            norm_chunk(2)
            oproj_chunk(2)
            import os

            if os.environ.get("K2_NO_EPI") != "1":
                epilogue()

    split_excess_waits(nc)
    return nc


TRACE = False
LAST_EXEC_NS = None

_NC = None


def _get_nc():
    global _NC
    if _NC is None:
        _NC = build()
    return _NC


def _qk_perm():
    """perm[j, c] = flat qk index (head*64 + dh) for psum partition c of
    projection block j (j = 2g + i)."""
    perm = np.zeros((8, 128), np.int64)
    for j in range(8):
        g, i = (j % 4) // 2, j % 2
        for c in range(128):
            m = 4 * g + c // 32
            dh = 32 * i + c % 32
            perm[j, c] = 64 * m + dh
    return perm


def kernel(x, W_qkv, b_qkv, W_o, b_o):
    x = np.asarray(x, dtype=np.float32)
    W_qkv = np.asarray(W_qkv, dtype=np.float32)
    b_qkv = np.asarray(b_qkv, dtype=np.float32)
    W_o = np.asarray(W_o, dtype=np.float32)
    b_o = np.asarray(b_o, dtype=np.float32)
    import ml_dtypes

    BF = ml_dtypes.bfloat16
    E4 = ml_dtypes.float8_e4m3

    # x8 per batch: [128, th, u, i, 1024] with d = 256u + 128i + p
    x8s = []
    for b in range(B):
        xT = x[b].T.reshape(4, 2, 128, 2, 1024)     # [u, i, p, th, t]
        x8s.append(
            np.ascontiguousarray(
                xT.transpose(2, 3, 0, 1, 4).reshape(128, 16384).astype(E4)
            )
        )

    PERM = _qk_perm()

    # triangular Schraudolph bias [128, 2 heads, 128] f32
    qq = np.arange(128)[None, :]
    pp = np.arange(128)[:, None]
    tri = np.where(qq >= pp, np.float32(SCH_B8), np.float32(SCH_MASKED8))
    bmask = np.ascontiguousarray(
        np.stack([tri, tri], axis=1).reshape(128, 256).astype(np.float32)
    )
    tri16 = np.where(qq >= pp, np.float32(SCH_B16), np.float32(SCH_MASKED16))
    bmaske = np.ascontiguousarray(
        np.stack([tri16, tri16], axis=1).reshape(128, 256).astype(np.float32)
    )

    # epilogue bf16 x slice per batch: [128, dt, t] for t 0..127
    xbfs = []
    for b in range(B):
        xe = x[b][0:128].T.reshape(8, 128, 128).transpose(1, 0, 2)
        xbfs.append(np.ascontiguousarray(xe.reshape(128, 1024).astype(BF)))

    in_maps = []
    for c in range(N_CORES):
        b, g = divmod(c, 2)
        h0 = g * HPC
        qcols = W_qkv[:, h0 * DH : h0 * DH + DPC]
        kcols = W_qkv[:, D + h0 * DH : D + h0 * DH + DPC]
        vcols = W_qkv[:, 2 * D + h0 * DH : 2 * D + h0 * DH + DPC]
        bq = b_qkv[h0 * DH : h0 * DH + DPC]
        bk = b_qkv[D + h0 * DH : D + h0 * DH + DPC]
        bvv = b_qkv[2 * D + h0 * DH : 2 * D + h0 * DH + DPC]

        # wqk8 [p, j, u, i, c]: W[d = 256u+128i+p, 128j+c] (flat qk order)
        wqk = np.concatenate([qcols, kcols], axis=1)  # [1024, 1024]
        wqk = wqk.reshape(4, 2, 128, 8, 128).transpose(2, 3, 0, 1, 4)
        wqk8 = np.ascontiguousarray(wqk.reshape(128, 8192).astype(E4))

        # wv8 [p, u, i, v]
        wv = vcols.reshape(4, 2, 128, 512).transpose(2, 0, 1, 3)
        wv8 = np.ascontiguousarray(wv.reshape(128, 4096).astype(E4))

        # wo8 [p, g, i, outcol]: row odim = 64*(4g+2i+p//64) + p%64
        wo8 = np.zeros((128, 2, 2, 1024), np.float32)
        wo_rows = W_o[g * DPC : (g + 1) * DPC, :]    # [512, 1024]
        for gg in range(2):
            for i in range(2):
                for ph in range(2):
                    m = 4 * gg + 2 * i + ph
                    wo8[64 * ph : 64 * ph + 64, gg, i, :] = wo_rows[
                        64 * m : 64 * m + 64, :
                    ]
        wo8 = np.ascontiguousarray(wo8.reshape(128, 4096).astype(E4))

        bqk = np.zeros((128, 8), np.float32)
        for j in range(4):
            bqk[:, j] = bq[128 * j : 128 * (j + 1)]
            bqk[:, 4 + j] = bk[128 * j : 128 * (j + 1)]

        # bf16 epilogue weights (plain 128-block layouts, no strips)
        wqkbf = (
            np.concatenate([qcols, kcols], axis=1)
            .reshape(8, 128, 8, 128)
            .transpose(1, 2, 0, 3)
            .reshape(128, 8192)
            .astype(BF)
        )
        wvbf = (
            vcols.reshape(8, 128, 512).transpose(1, 0, 2).reshape(128, 4096)
        ).astype(BF)
        wobf = (
            W_o[g * DPC : (g + 1) * DPC, :]
            .reshape(4, 128, D)
            .transpose(1, 0, 2)
            .reshape(128, 4 * D)
        ).astype(BF)
        bqke = np.zeros((128, 8), np.float32)
        for j in range(4):
            bqke[:, j] = bq[128 * j : 128 * (j + 1)]
            bqke[:, 4 + j] = bk[128 * j : 128 * (j + 1)]

        in_maps.append(
            {
                "x8": x8s[b],
                "wqk8": wqk8,
                "wv8": wv8,
                "wo8": wo8,
                "bqk": np.ascontiguousarray(bqk),
                "bv": np.ascontiguousarray(bvv.astype(BF).reshape(1, DPC)),
                "bmask8": bmask,
                "xbf": xbfs[b],
                "wqkbf": np.ascontiguousarray(wqkbf),
                "wvbf": np.ascontiguousarray(wvbf),
                "wobf": np.ascontiguousarray(wobf),
                "bqke": np.ascontiguousarray(bqke),
                "bmaske": bmaske,
            }
        )

    nc = _get_nc()
    global LAST_EXEC_NS
    res = None
    last_err = None
    for attempt in range(3):
        try:
            res = run_bass_kernel_spmd(
                nc, in_maps, list(range(N_CORES)), trace=TRACE
            )
            break
        except Exception as e:  # transient device wedge: retry
            last_err = e
            import time as _time

            _time.sleep(5)
    if res is None:
        raise last_err
    LAST_EXEC_NS = res.exec_time_ns
    globals()["_LAST_RES"] = res
    parts = [
        res.results[c]["out"].astype(np.float32) for c in range(N_CORES)
    ]
    parts2 = [
        res.results[c]["out2"].astype(np.float32) for c in range(N_CORES)
    ]
    out = np.empty((B, T, D), np.float32)
    for b in range(B):
        out[b] = parts[2 * b] + parts[2 * b + 1] + b_o[None, :]
        # exact bf16 recompute of the fp8-noisy first 128 rows
        out[b, 0:128] = parts2[2 * b] + parts2[2 * b + 1] + b_o[None, :]
    return out


# revision 3
# speedup vs baseline: 1.0469x; 1.0469x over previous
"""Causal MHA (B=4, T=2048, D=1024, H=16) on 8 TRN2 NeuronCores — fp8 edition.

Sharding: core c -> batch b = c // 2, head-group g = c % 2 (8 heads each).
All five matmul families run fp8e4m3 DoubleRow (0.5 PE cycles/row):

  qkv   : x^T pre-packed [128, th, u, i, t] pairing d=256u+128i+p; W same.
  S     : Q,K stored [128, 2, T] fp8 — 4 heads per tile in 32-partition
          strips (strip 32*(m%4)), pair dim = dh//32.  1/sqrt(dh) folded
          into the exp scale, not into Q (fp8 subnormal hazard).
  AV    : probs written by exp as fp8 into [128, kpair, head, 512] tiles;
          V packed [128, kpair, head, 128] (col 64 = ones -> rowsums in
          psum; cols 65:128 zero pad — dual-fp8 ldweights wants full
          tiles).  One DoubleRow matmul contracts TWO k-tiles.
  oproj : onorm fp8 [128, 2, T] pairing head-pairs; W_o fp8 pairs.

exp: off-diagonal tiles alternate ScalarE Exp(scale=1/8)->fp8 and DVE
Schraudolph int8-saturation -> e4m3 bits.  Diagonal tiles split: DVE does
the 128-wide triangular block (mask folded into a f32 bias operand; -1e9
saturates int8 to 0x80 = -0.0), ScalarE the unmasked tail, GpSimd memsets
the dead prefix.  Row-sum reciprocal via DRAM-respread as baseline.
Output partials stored bf16; host sums pairs in f32 and adds b_o.
"""

import sys

sys.path.insert(0, "/opt/trn_rl_repo")

import numpy as np

import concourse.bass as bass
import concourse.mybir as mybir
from concourse.bass_utils import run_bass_kernel_spmd
from concourse.tile import TileContext

F32 = mybir.dt.float32
BF16 = mybir.dt.bfloat16
FP8 = mybir.dt.float8e4
I8 = mybir.dt.int8
I16 = mybir.dt.int16
EXP = mybir.ActivationFunctionType.Exp
COPY = mybir.ActivationFunctionType.Copy
MULT = mybir.AluOpType.mult
ADD = mybir.AluOpType.add
DR = mybir.MatmulPerfMode.DoubleRow

B, T, D, H = 4, 2048, 1024, 16
DH = D // H          # 64
HPC = H // 2         # heads per core = 8
DPC = HPC * DH       # 512 projected dims per core
N_CORES = 8
QC = 512             # q-chunk width

SCH_A8 = 1.4426950408889634   # exp(s/8): (1/8) * 8*log2(e)
SCH_B8 = 56.0 - 0.4625        # 7*8 with mean-centering correction
SCH_MASKED8 = SCH_B8 - 1.0e9  # saturates int8 -> -128 -> e4m3 -0.0
SCH_A16 = 184.66496030 / 8.0  # bf16 Schraudolph with /8 folded in
SCH_B16 = 16256.0 - 7.4
SCH_MASKED16 = SCH_B16 - 1.0e9


def split_excess_waits(nc, cap=1):
    """walrus limits sync-wait slots per ISA instruction (1 for several
    structs).  Move excess waits onto InstEventSemaphore instructions
    inserted just before the offender on the same engine."""
    n_split = 0
    for f in nc.m.functions:
        for blk in f.blocks:
            insts = blk.instructions
            out = []
            changed = False
            for inst in insts:
                si = inst.sync_info
                waits = list(si.on_wait) if si is not None else []
                if len(waits) > cap:
                    for j, w in enumerate(waits[:-cap]):
                        ev = mybir.InstEventSemaphore(
                            name=f"{inst.name}-w{j}", ins=[], outs=[]
                        )
                        ev.engine = inst.engine
                        ev.sync_info = mybir.SyncInfo(on_wait=[w], on_update=[])
                        out.append(ev)
                        n_split += 1
                    inst.sync_info = mybir.SyncInfo(
                        on_wait=waits[-cap:], on_update=list(si.on_update)
                    )
                    changed = True
                out.append(inst)
            if changed:
                blk.instructions = out
    return n_split


INST_LABELS = {}


def build():
    nc = bass.Bass(target_bir_lowering=False)

    _label = ["init"]

    def set_label(s):
        _label[0] = s

    for eng in (nc.tensor, nc.vector, nc.scalar, nc.gpsimd, nc.sync):
        orig = eng.add_instruction

        def wrapped(inst, _orig=orig):
            r = _orig(inst)
            try:
                INST_LABELS[inst.name] = _label[0]
            except Exception:
                pass
            return r

        eng.add_instruction = wrapped

    # x8: [128, th, u, i, tcol]  (d = 256u + 128i + p, t = 1024*th + tcol)
    x8_d = nc.dram_tensor("x8", [128, 2 * 4 * 2 * 1024], FP8, kind="ExternalInput")
    # wqk8: [p, j, u, i, c]  j=0..3 Q (j=2g+iq), j=4..7 K
    wqk8_d = nc.dram_tensor("wqk8", [128, 8 * 4 * 2 * 128], FP8, kind="ExternalInput")
    # wv8: [p, u, i, v]
    wv8_d = nc.dram_tensor("wv8", [128, 4 * 2 * 512], FP8, kind="ExternalInput")
    # wo8: [p, g, i, outcol]
    wo8_d = nc.dram_tensor("wo8", [128, 2 * 2 * 1024], FP8, kind="ExternalInput")
    bqk_d = nc.dram_tensor("bqk", [128, 8], F32, kind="ExternalInput")
    bv_d = nc.dram_tensor("bv", [1, DPC], BF16, kind="ExternalInput")
    # trimask bias: [p, head, 128] f32 (keep: SCH_B8, masked: SCH_B8-1e9)
    bmask_d = nc.dram_tensor("bmask8", [128, 2 * 128], F32, kind="ExternalInput")
    # ---- bf16 epilogue inputs (exact recompute of rows 0..127) ----
    xbf_d = nc.dram_tensor("xbf", [128, 8 * 128], BF16, kind="ExternalInput")
    wqkbf_d = nc.dram_tensor("wqkbf", [128, 8 * 8 * 128], BF16, kind="ExternalInput")
    wvbf_d = nc.dram_tensor("wvbf", [128, 8 * 512], BF16, kind="ExternalInput")
    wobf_d = nc.dram_tensor("wobf", [128, 4 * 1024], BF16, kind="ExternalInput")
    bqke_d = nc.dram_tensor("bqke", [128, 8], F32, kind="ExternalInput")
    bmaske_d = nc.dram_tensor("bmaske", [128, 2 * 128], F32, kind="ExternalInput")
    out_d = nc.dram_tensor("out", [T, D], BF16, kind="ExternalOutput")
    out2_d = nc.dram_tensor("out2", [128, D], BF16, kind="ExternalOutput")
    rsraw_d = nc.dram_tensor("rsraw", [4, 4096], BF16)  # raw rowsums / chunk
    rrec_d = nc.dram_tensor("rrec", [4, 4096], BF16)    # reciprocals / chunk
    rsraw2_d = nc.dram_tensor("rsraw2", [1, 1024], BF16)
    rrec2_d = nc.dram_tensor("rrec2", [1, 1024], BF16)

    with TileContext(nc) as tc:
        with (
            tc.tile_pool(name="const", bufs=1) as constp,
            tc.tile_pool(name="wstream", bufs=8) as wp,
            tc.tile_pool(name="xt", bufs=2) as xtp,
            tc.tile_pool(name="qk", bufs=1) as qkp,
            tc.tile_pool(name="vaug", bufs=1) as vp,
            tc.tile_pool(name="onorm", bufs=1) as onp,
            tc.tile_pool(name="pt", bufs=4) as ptp,
            tc.tile_pool(name="sc", bufs=8) as scp,
            tc.tile_pool(name="rs", bufs=4) as rsp,
            tc.tile_pool(name="bc", bufs=8) as bcp,
            tc.tile_pool(name="osb", bufs=3) as osbp,
            tc.tile_pool(name="ps", bufs=4, space="PSUM") as psp,
            tc.tile_pool(name="spair", bufs=2, space="PSUM") as spp,
        ):
            # qk_sb[j]: bf16 [128, T], heads pair-stacked (64+64) as in the
            # bf16 baseline — S matmuls gain nothing from dual-fp8 (the win
            # is per-row contraction depth, and S only contracts 64), so
            # keep them bf16 with concurrent PE quadrants.
            qk_sb = [
                qkp.tile([128, T], BF16, tag=f"qk{j}", name=f"qk{j}")
                for j in range(8)
            ]
            # vaug2[tp]: [128, 2, 8, 128]  (k = 256*tp+128*i+p, head, cols)
            vaug2 = [
                vp.tile([128, 2, HPC, 2 * DH], FP8, tag=f"v{t}", name=f"v{t}")
                for t in range(8)
            ]
            # onorm8[g]: [128, 2, T]  (odim = 64*(4g+2i+p//64) + p%64)
            onorm8 = [
                onp.tile([128, 2, T], FP8, tag=f"on{g}", name=f"on{g}")
                for g in range(2)
            ]

            set_label("qkv")

            def load_wj(th, j):
                w_j = wp.tile([128, 4, 2, 128], FP8, tag="wqk", name=f"w{th}_{j}")
                nc.sync.dma_start(
                    w_j[:],
                    wqk8_d[:, 1024 * j : 1024 * (j + 1)].rearrange(
                        "p (u i c) -> p u i c", u=4, i=2
                    ),
                )
                return w_j

            xts = {}

            def load_x(th):
                xt = []
                for u in range(4):
                    x_t = xtp.tile(
                        [128, 2, 1024], FP8, tag=f"xt{u}", name=f"xt{th}_{u}"
                    )
                    eng = nc.sync if u % 2 == 0 else nc.gpsimd
                    off = (th * 4 + u) * 2048
                    eng.dma_start(
                        x_t[:],
                        x8_d[:, off : off + 2048].rearrange(
                            "p (i c) -> p i c", i=2
                        ),
                    )
                    xt.append(x_t)
                xts[th] = xt

            # ---- first loads: w(j=0) + x(th0) before any consts ----
            w_first = load_wj(0, 0)
            load_x(0)

            set_label("const")
            wv_sb = constp.tile([128, 4, 2, 512], FP8, tag="wv")
            nc.gpsimd.dma_start(
                wv_sb[:], wv8_d[:].rearrange("p (u i v) -> p u i v", u=4, i=2)
            )
            bqk_sb = constp.tile([128, 8], F32, tag="bqk")
            nc.gpsimd.dma_start(bqk_sb[:], bqk_d[:])
            # V bias broadcast to all partitions (DVE add, no bias matmul)
            bvb_sb = constp.tile([128, 8, DH], BF16, tag="bvb")
            nc.gpsimd.dma_start(
                bvb_sb[:].rearrange("p h d -> p (h d)"),
                bass.AP(bv_d, 0, [[0, 128], [1, DPC]]),
            )
            bmask_sb = constp.tile([128, 2, 128], F32, tag="bmask")
            nc.gpsimd.dma_start(
                bmask_sb[:],
                bmask_d[:].rearrange("p (h q) -> p h q", h=2),
            )
            # wo early: oproj chunk 0 runs during t-half-1 projections
            wo_sb = constp.tile([128, 2, 2, 1024], FP8, tag="wo")
            nc.gpsimd.dma_start(
                wo_sb[:], wo8_d[:].rearrange("p (g i c) -> p g i c", g=2, i=2)
            )
            # zero the pad cols of vaug so psum garbage rows stay finite
            for tp_i in range(8):
                nc.gpsimd.memset(vaug2[tp_i][:, :, :, DH + 1 :], 0.0)

            # bf16 epilogue constants (rows 0..127 exact recompute)
            xbf_sb = constp.tile([128, 8, 128], BF16, tag="xbf")
            nc.gpsimd.dma_start(
                xbf_sb[:], xbf_d[:].rearrange("p (d t) -> p d t", d=8)
            )
            wqkbf_sb = constp.tile([128, 8, 8, 128], BF16, tag="wqkbf")
            nc.gpsimd.dma_start(
                wqkbf_sb[:],
                wqkbf_d[:].rearrange("p (j d c) -> p j d c", j=8, d=8),
            )
            wvbf_sb = constp.tile([128, 8, 512], BF16, tag="wvbf")
            nc.gpsimd.dma_start(
                wvbf_sb[:], wvbf_d[:].rearrange("p (d v) -> p d v", d=8)
            )
            wobf_sb = constp.tile([128, 4, 1024], BF16, tag="wobf")
            nc.gpsimd.dma_start(
                wobf_sb[:], wobf_d[:].rearrange("p (h c) -> p h c", h=4)
            )
            bqke_sb = constp.tile([128, 8], F32, tag="bqke")
            nc.gpsimd.dma_start(bqke_sb[:], bqke_d[:])
            bmaske_sb = constp.tile([128, 2, 128], F32, tag="bmaske")
            nc.gpsimd.dma_start(
                bmaske_sb[:], bmaske_d[:].rearrange("p (h q) -> p h q", h=2)
            )

            # ---- phase 1: projections for one t-half ----
            def proj_half(th):
                t0 = th * (T // 2)
                xt = xts[th]
                set_label("qkv")
                w_tiles = [
                    w_first if (th == 0 and j == 0) else load_wj(th, j)
                    for j in range(8)
                ]
                for j in range(8):
                    w_j = w_tiles[j]
                    ps0 = psp.tile([128, 512], F32, tag="ps", name=f"q{th}{j}a")
                    ps1 = psp.tile([128, 512], F32, tag="ps", name=f"q{th}{j}b")
                    for u in range(4):
                        nc.tensor.matmul(
                            ps0[:],
                            w_j[:, u, :, :],
                            xt[u][:, :, 0:512],
                            start=(u == 0),
                            stop=(u == 3),
                            perf_mode=DR,
                        )
                        nc.tensor.matmul(
                            ps1[:],
                            w_j[:, u, :, :],
                            xt[u][:, :, 512:1024],
                            start=(u == 0),
                            stop=(u == 3),
                            perf_mode=DR,
                        )
                    nc.vector.tensor_scalar_add(
                        qk_sb[j][:, t0 : t0 + 512], ps0[:], bqk_sb[:, j : j + 1]
                    )
                    nc.vector.tensor_scalar_add(
                        qk_sb[j][:, t0 + 512 : t0 + 1024],
                        ps1[:],
                        bqk_sb[:, j : j + 1],
                    )

                # V: bias via DVE broadcast-add; ones col via memset
                set_label("vproj")
                for tt in range(8):
                    tg = th * 8 + tt
                    tp_i, i = tg // 2, tg % 2
                    ps = psp.tile([128, 512], F32, tag="ps", name=f"v{th}{tt}")
                    for u in range(4):
                        nc.tensor.matmul(
                            ps[:],
                            xt[u][:, :, 128 * tt : 128 * (tt + 1)],
                            wv_sb[:, u, :, :],
                            start=(u == 0),
                            stop=(u == 3),
                            perf_mode=DR,
                        )
                    nc.vector.tensor_tensor(
                        vaug2[tp_i][:, i, :, 0:DH],
                        ps[:].rearrange("p (h d) -> p h d", h=HPC),
                        bvb_sb[:],
                        ADD,
                    )
                    nc.gpsimd.memset(vaug2[tp_i][:, i, :, DH : DH + 1], 1.0)

            # ---- phase 2: attention ----
            _off_rr = [0]

            def emit_exp(c, t, qsP, sp, pt, i_p):
                """exp of k-tile t's scores (both heads) into pt[:, i_p].
                The pair covers cols [qsP:512]."""
                j = t - 4 * c
                spv = sp[:].rearrange("p (h q) -> p h q", h=2)
                if j < 0:
                    # fully off-diagonal: whole-tile instructions amortize
                    # the ~350ns per-instruction overhead; rotate ACT:DVE
                    # 5:4 to equalize total engine load
                    eng = "act" if _off_rr[0] % 9 < 5 else "dve"
                    _off_rr[0] += 1
                    ptv = pt[:, i_p, :, :]
                    if eng == "act":
                        nc.scalar.activation(ptv, spv, EXP, scale=0.125)
                    else:
                        nc.vector.tensor_scalar(
                            ptv.bitcast(I8), spv, SCH_A8, SCH_B8, MULT, ADD
                        )
                    return
                qs = 128 * j
                if qs > qsP:
                    # dead prefix of the second diag tile of the pair
                    nc.gpsimd.memset(pt[:, i_p, :, qsP:qs], 0.0)
                # triangular block on DVE (mask folded into f32 bias)
                nc.vector.scalar_tensor_tensor(
                    pt[:, i_p, :, qs : qs + 128].bitcast(I8),
                    spv[:, :, qs : qs + 128],
                    SCH_A8,
                    bmask_sb[:],
                    MULT,
                    ADD,
                )
                # unmasked tail on ACT
                if qs + 128 < 512:
                    nc.scalar.activation(
                        pt[:, i_p, :, qs + 128 : 512],
                        spv[:, :, qs + 128 : 512],
                        EXP,
                        scale=0.125,
                    )

            _chunk_sc = {}

            def attn_block2(c, hpA, hpB):
                """two head-pairs interleaved: while one stream's exp runs,
                the other stream's S/AV matmuls keep every engine fed."""
                set_label("attn")
                q0 = QC * c
                ktiles = 4 * (c + 1)
                oPS = {
                    hp: [
                        psp.tile([128, 512], F32, tag="ps", name=f"oA{c}{hp}"),
                        psp.tile([128, 512], F32, tag="ps", name=f"oB{c}{hp}"),
                    ]
                    for hp in (hpA, hpB)
                }

                def s_mm(hp, t):
                    j = t - 4 * c
                    qs = 128 * j if j >= 0 else 0
                    sp = spp.tile([128, 1024], F32, tag="sp", name=f"sp{hp}_{t}")
                    for half, base in ((0, 0), (1, 64)):
                        nc.tensor.matmul(
                            sp[:, 512 * half + qs : 512 * (half + 1)],
                            qk_sb[4 + hp][base : base + 64, 128 * t : 128 * (t + 1)],
                            qk_sb[hp][base : base + 64, q0 + qs : q0 + QC],
                            start=True,
                            stop=True,
                            tile_position=(base, 0),
                        )
                    return sp

                def av_mm(hp, tp_i, qsP, pt, last):
                    for half in range(2):
                        hh = 2 * hp + half
                        nc.tensor.matmul(
                            oPS[hp][half][:, qsP:512],
                            vaug2[tp_i][:, :, hh, :],
                            pt[:, :, half, qsP:512],
                            start=(tp_i == 0),
                            stop=last,
                            perf_mode=DR,
                            skip_group_check=True,
                        )

                npairs = ktiles // 2
                for tp_i in range(npairs):
                    t0_, t1_ = 2 * tp_i, 2 * tp_i + 1
                    j0 = t0_ - 4 * c
                    qsP = 128 * j0 if j0 >= 0 else 0
                    ptd = {}
                    for hp in (hpA, hpB):
                        sp0 = s_mm(hp, t0_)
                        sp1 = s_mm(hp, t1_)
                        pt = ptp.tile(
                            [128, 2, 2, 512], FP8, tag="pt", name=f"pt{hp}_{t0_}"
                        )
                        emit_exp(c, t0_, qsP, sp0, pt, 0)
                        emit_exp(c, t1_, qsP, sp1, pt, 1)
                        ptd[hp] = pt
                    for hp in (hpA, hpB):
                        av_mm(hp, tp_i, qsP, ptd[hp], last=(tp_i == npairs - 1))

                # evict raw o + rowsum row together (ACT, one [65,512] copy);
                # rowsum row DMAs to DRAM for the chunk-batched respread
                set_label("norm")
                for hp in (hpA, hpB):
                    for half in range(2):
                        idx = 2 * hp + half
                        sc = scp.tile(
                            [DH + 1, 512], BF16, tag="sc", name=f"sc{c}{hp}{half}"
                        )
                        nc.scalar.activation(
                            sc[:], oPS[hp][half][0 : DH + 1, :], COPY
                        )
                        nc.gpsimd.dma_start(
                            rsraw_d[c : c + 1, idx * 512 : (idx + 1) * 512],
                            sc[DH : DH + 1, :],
                        )
                        _chunk_sc[(c, hp, half)] = sc

            def norm_half(c, hpg):
                """batched reciprocal + normalize for head-pairs of one
                block2 (hps 2*hpg, 2*hpg+1) — overlaps the next block2"""
                set_label("norm")
                base = c * 4096 + hpg * 2048
                rload = rsp.tile([128, 16], BF16, tag="rload")
                nc.sync.dma_start(
                    rload[:], bass.AP(rsraw_d, base, [[16, 128], [1, 16]])
                )
                rrec = rsp.tile([128, 16], BF16, tag="rrec")
                with nc.allow_low_precision(
                    reason="bf16 softmax-normalizer reciprocal; well inside "
                    "the output tolerance"
                ):
                    nc.vector.reciprocal(rrec[:], rload[:])
                nc.sync.dma_start(
                    bass.AP(rrec_d, base, [[16, 128], [1, 16]]), rrec[:]
                )
                for hp in (2 * hpg, 2 * hpg + 1):
                    for half in range(2):
                        m = 2 * hp + half
                        gg, i_o = m // 4, (m % 4) // 2
                        bc = bcp.tile([64, 512], BF16, tag="bc")
                        nc.sync.dma_start(
                            bc[:],
                            bass.AP(
                                rrec_d,
                                c * 4096 + m * 512,
                                [[0, 64], [1, 512]],
                            ),
                        )
                        sc = _chunk_sc.pop((c, hp, half))
                        nc.gpsimd.tensor_tensor(
                            onorm8[gg][
                                64 * (m % 2) : 64 * (m % 2) + 64,
                                i_o,
                                QC * c : QC * (c + 1),
                            ],
                            sc[0:DH, :],
                            bc[:],
                            MULT,
                        )

            def attn_chunk(c):
                attn_block2(c, 0, 1)
                norm_half(c, 0)
                attn_block2(c, 2, 3)
                norm_half(c, 1)

            def oproj_chunk(c):
                oproj_qts(4 * c, 4 * c + 4)

            # ---- phase 3: output projection ----
            def oproj_qts(q0_, q1_):
                set_label("oproj")
                for qt in range(q0_, q1_):
                    ps0 = psp.tile([128, 512], F32, tag="ps", name=f"o{qt}a")
                    ps1 = psp.tile([128, 512], F32, tag="ps", name=f"o{qt}b")
                    for gg in range(2):
                        nc.tensor.matmul(
                            ps0[:],
                            onorm8[gg][:, :, 128 * qt : 128 * (qt + 1)],
                            wo_sb[:, gg, :, 0:512],
                            start=(gg == 0),
                            stop=(gg == 1),
                            perf_mode=DR,
                        )
                        nc.tensor.matmul(
                            ps1[:],
                            onorm8[gg][:, :, 128 * qt : 128 * (qt + 1)],
                            wo_sb[:, gg, :, 512:1024],
                            start=(gg == 0),
                            stop=(gg == 1),
                            perf_mode=DR,
                        )
                    for dc, ps in ((0, ps0), (1, ps1)):
                        osb = osbp.tile([128, 512], BF16, tag="osb")
                        nc.scalar.activation(osb[:], ps[:], COPY)
                        nc.gpsimd.dma_start(
                            out_d[
                                128 * qt : 128 * (qt + 1),
                                512 * dc : 512 * (dc + 1),
                            ],
                            osb[:],
                        )

            # ---- bf16 epilogue: exact recompute of output rows 0..127 ----
            def epilogue():
                set_label("epi")
                # Q,K: two psum tiles, 4 j-blocks each (col range 128*(j%4))
                qk_ep = constp.tile([128, 8, 128], BF16, tag="qk_ep")
                for grp in range(2):  # 0 = Q blocks 0..3, 1 = K blocks 4..7
                    ps = psp.tile([128, 512], F32, tag="ps", name=f"eqk{grp}")
                    for jj in range(4):
                        j = 4 * grp + jj
                        for dt in range(8):
                            nc.tensor.matmul(
                                ps[:, 128 * jj : 128 * (jj + 1)],
                                wqkbf_sb[:, j, dt, :],
                                xbf_sb[:, dt, :],
                                start=(dt == 0),
                                stop=(dt == 7),
                                skip_group_check=True,
                            )
                    for jj in range(4):
                        j = 4 * grp + jj
                        nc.vector.tensor_scalar_add(
                            qk_ep[:, j, :],
                            ps[:, 128 * jj : 128 * (jj + 1)],
                            bqke_sb[:, j : j + 1],
                        )
                # V: [128 t, 512 v] + bias + ones col
                vep = constp.tile([128, HPC, DH + 1], BF16, tag="vep")
                psv = psp.tile([128, 512], F32, tag="ps", name="evps")
                for dt in range(8):
                    nc.tensor.matmul(
                        psv[:],
                        xbf_sb[:, dt, :],
                        wvbf_sb[:, dt, :],
                        start=(dt == 0),
                        stop=(dt == 7),
                    )
                nc.vector.tensor_tensor(
                    vep[:, :, 0:DH],
                    psv[:].rearrange("p (h d) -> p h d", h=HPC),
                    bvb_sb[:],
                    ADD,
                )
                nc.gpsimd.memset(vep[:, :, DH : DH + 1], 1.0)
                # attention per head pair (single diagonal k-tile)
                onorm_ep = constp.tile([128, 4, 128], BF16, tag="onorm_ep")
                for hp in range(4):
                    spE = spp.tile([128, 1024], F32, tag="sp", name=f"esp{hp}")
                    for half in range(2):
                        # halves in different psum banks (cols 0 / 512)
                        nc.tensor.matmul(
                            spE[:, 512 * half : 512 * half + 128],
                            qk_ep[64 * half : 64 * half + 64, 4 + hp, :],
                            qk_ep[64 * half : 64 * half + 64, hp, :],
                            start=True,
                            stop=True,
                            tile_position=(64 * half, 0),
                        )
                    ptE = bcp.tile([128, 2, 128], BF16, tag="ptE", name=f"ept{hp}")
                    nc.vector.scalar_tensor_tensor(
                        ptE[:].bitcast(I16),
                        spE[:].rearrange("p (h q) -> p h q", h=2)[:, :, 0:128],
                        SCH_A16,
                        bmaske_sb[:],
                        MULT,
                        ADD,
                    )
                    oE = psp.tile([128, 512], F32, tag="ps", name=f"eo{hp}")
                    for half in range(2):
                        nc.tensor.matmul(
                            oE[0 : DH + 1, 128 * half : 128 * (half + 1)],
                            vep[:, 2 * hp + half, :],
                            ptE[:, half, :],
                            start=True,
                            stop=True,
                            skip_group_check=True,
                        )
                    # rowsums for both halves -> DRAM respread
                    rrowE = rsp.tile([1, 256], BF16, tag="rrowE")
                    nc.vector.tensor_copy(
                        out=rrowE[:], in_=oE[DH : DH + 1, 0:256]
                    )
                    nc.gpsimd.dma_start(
                        rsraw2_d[0:1, 256 * hp : 256 * (hp + 1)], rrowE[:]
                    )
                    scE = scp.tile([64, 256], BF16, tag="scE", name=f"scE{hp}")
                    nc.scalar.activation(scE[:], oE[0:DH, 0:256], COPY)
                    rloadE = rsp.tile([128, 2], BF16, tag="rloadE")
                    nc.sync.dma_start(
                        rloadE[:],
                        bass.AP(rsraw2_d, 256 * hp, [[2, 128], [1, 2]]),
                    )
                    rrecE = rsp.tile([128, 2], BF16, tag="rrecE")
                    with nc.allow_low_precision(
                        reason="bf16 softmax-normalizer reciprocal"
                    ):
                        nc.vector.reciprocal(rrecE[:], rloadE[:])
                    nc.sync.dma_start(
                        bass.AP(rrec2_d, 256 * hp, [[2, 128], [1, 2]]),
                        rrecE[:],
                    )
                    for half in range(2):
                        bcE = bcp.tile([64, 128], BF16, tag="bcE")
                        nc.sync.dma_start(
                            bcE[:],
                            bass.AP(
                                rrec2_d,
                                256 * hp + 128 * half,
                                [[0, 64], [1, 128]],
                            ),
                        )
                        nc.vector.tensor_tensor(
                            onorm_ep[64 * half : 64 * half + 64, hp, :],
                            scE[:, 128 * half : 128 * (half + 1)],
                            bcE[:],
                            MULT,
                        )
                # oproj for the 128 rows
                psa = psp.tile([128, 512], F32, tag="ps", name="eoa")
                psb = psp.tile([128, 512], F32, tag="ps", name="eob")
                for hp in range(4):
                    nc.tensor.matmul(
                        psa[:],
                        onorm_ep[:, hp, :],
                        wobf_sb[:, hp, 0:512],
                        start=(hp == 0),
                        stop=(hp == 3),
                    )
                    nc.tensor.matmul(
                        psb[:],
                        onorm_ep[:, hp, :],
                        wobf_sb[:, hp, 512:1024],
                        start=(hp == 0),
                        stop=(hp == 3),
                    )
                for dc, ps in ((0, psa), (1, psb)):
                    osb = osbp.tile([128, 512], BF16, tag="osb")
                    nc.scalar.activation(osb[:], ps[:], COPY)
                    nc.gpsimd.dma_start(
                        out2_d[:, 512 * dc : 512 * (dc + 1)], osb[:]
                    )

            # ---- emission order ----
            proj_half(0)
            load_x(1)          # prefetch t-half-1 x during early attention
            attn_chunk(0)
            epilogue()         # hides under chunk-1 attention
            attn_chunk(1)
            oproj_chunk(0)
            proj_half(1)
            oproj_chunk(1)
            attn_chunk(3)      # big chunk first: its oproj hides under a2
            attn_block2(2, 0, 1)
            oproj_qts(12, 14)  # chunk-3 oproj interleaved into chunk 2
            norm_half(2, 0)
            attn_block2(2, 2, 3)
            oproj_qts(14, 16)
            norm_half(2, 1)
            oproj_chunk(2)

    split_excess_waits(nc)
    return nc


TRACE = False
LAST_EXEC_NS = None

_NC = None


def _get_nc():
    global _NC
    if _NC is None:
        _NC = build()
    return _NC


def _qk_perm():
    """perm[j, c] = flat qk index (head*64 + dh) for psum partition c of
    projection block j (j = 2g + i)."""
    perm = np.zeros((8, 128), np.int64)
    for j in range(8):
        g, i = (j % 4) // 2, j % 2
        for c in range(128):
            m = 4 * g + c // 32
            dh = 32 * i + c % 32
            perm[j, c] = 64 * m + dh
    return perm


def kernel(x, W_qkv, b_qkv, W_o, b_o):
    x = np.asarray(x, dtype=np.float32)
    W_qkv = np.asarray(W_qkv, dtype=np.float32)
    b_qkv = np.asarray(b_qkv, dtype=np.float32)
    W_o = np.asarray(W_o, dtype=np.float32)
    b_o = np.asarray(b_o, dtype=np.float32)
    import ml_dtypes

    BF = ml_dtypes.bfloat16
    E4 = ml_dtypes.float8_e4m3

    # x8 per batch: [128, th, u, i, 1024] with d = 256u + 128i + p
    x8s = []
    for b in range(B):
        xT = x[b].T.reshape(4, 2, 128, 2, 1024)     # [u, i, p, th, t]
        x8s.append(
            np.ascontiguousarray(
                xT.transpose(2, 3, 0, 1, 4).reshape(128, 16384).astype(E4)
            )
        )

    PERM = _qk_perm()

    # triangular Schraudolph bias [128, 2 heads, 128] f32
    qq = np.arange(128)[None, :]
    pp = np.arange(128)[:, None]
    tri = np.where(qq >= pp, np.float32(SCH_B8), np.float32(SCH_MASKED8))
    bmask = np.ascontiguousarray(
        np.stack([tri, tri], axis=1).reshape(128, 256).astype(np.float32)
    )
    tri16 = np.where(qq >= pp, np.float32(SCH_B16), np.float32(SCH_MASKED16))
    bmaske = np.ascontiguousarray(
        np.stack([tri16, tri16], axis=1).reshape(128, 256).astype(np.float32)
    )

    # epilogue bf16 x slice per batch: [128, dt, t] for t 0..127
    xbfs = []
    for b in range(B):
        xe = x[b][0:128].T.reshape(8, 128, 128).transpose(1, 0, 2)
        xbfs.append(np.ascontiguousarray(xe.reshape(128, 1024).astype(BF)))

    in_maps = []
    for c in range(N_CORES):
        b, g = divmod(c, 2)
        h0 = g * HPC
        qcols = W_qkv[:, h0 * DH : h0 * DH + DPC]
        kcols = W_qkv[:, D + h0 * DH : D + h0 * DH + DPC]
        vcols = W_qkv[:, 2 * D + h0 * DH : 2 * D + h0 * DH + DPC]
        bq = b_qkv[h0 * DH : h0 * DH + DPC]
        bk = b_qkv[D + h0 * DH : D + h0 * DH + DPC]
        bvv = b_qkv[2 * D + h0 * DH : 2 * D + h0 * DH + DPC]

        # wqk8 [p, j, u, i, c]: W[d = 256u+128i+p, 128j+c] (flat qk order)
        wqk = np.concatenate([qcols, kcols], axis=1)  # [1024, 1024]
        wqk = wqk.reshape(4, 2, 128, 8, 128).transpose(2, 3, 0, 1, 4)
        wqk8 = np.ascontiguousarray(wqk.reshape(128, 8192).astype(E4))

        # wv8 [p, u, i, v]
        wv = vcols.reshape(4, 2, 128, 512).transpose(2, 0, 1, 3)
        wv8 = np.ascontiguousarray(wv.reshape(128, 4096).astype(E4))

        # wo8 [p, g, i, outcol]: row odim = 64*(4g+2i+p//64) + p%64
        wo8 = np.zeros((128, 2, 2, 1024), np.float32)
        wo_rows = W_o[g * DPC : (g + 1) * DPC, :]    # [512, 1024]
        for gg in range(2):
            for i in range(2):
                for ph in range(2):
                    m = 4 * gg + 2 * i + ph
                    wo8[64 * ph : 64 * ph + 64, gg, i, :] = wo_rows[
                        64 * m : 64 * m + 64, :
                    ]
        wo8 = np.ascontiguousarray(wo8.reshape(128, 4096).astype(E4))

        bqk = np.zeros((128, 8), np.float32)
        for j in range(4):
            bqk[:, j] = bq[128 * j : 128 * (j + 1)]
            bqk[:, 4 + j] = bk[128 * j : 128 * (j + 1)]

        # bf16 epilogue weights (plain 128-block layouts, no strips)
        wqkbf = (
            np.concatenate([qcols, kcols], axis=1)
            .reshape(8, 128, 8, 128)
            .transpose(1, 2, 0, 3)
            .reshape(128, 8192)
            .astype(BF)
        )
        wvbf = (
            vcols.reshape(8, 128, 512).transpose(1, 0, 2).reshape(128, 4096)
        ).astype(BF)
        wobf = (
            W_o[g * DPC : (g + 1) * DPC, :]
            .reshape(4, 128, D)
            .transpose(1, 0, 2)
            .reshape(128, 4 * D)
        ).astype(BF)
        bqke = np.zeros((128, 8), np.float32)
        for j in range(4):
            bqke[:, j] = bq[128 * j : 128 * (j + 1)]
            bqke[:, 4 + j] = bk[128 * j : 128 * (j + 1)]

        in_maps.append(
            {
                "x8": x8s[b],
                "wqk8": wqk8,
                "wv8": wv8,
                "wo8": wo8,
                "bqk": np.ascontiguousarray(bqk),
                "bv": np.ascontiguousarray(bvv.astype(BF).reshape(1, DPC)),
                "bmask8": bmask,
                "xbf": xbfs[b],
                "wqkbf": np.ascontiguousarray(wqkbf),
                "wvbf": np.ascontiguousarray(wvbf),
                "wobf": np.ascontiguousarray(wobf),
                "bqke": np.ascontiguousarray(bqke),
                "bmaske": bmaske,
            }
        )

    nc = _get_nc()
    global LAST_EXEC_NS
    res = None
    last_err = None
    for attempt in range(3):
        try:
            res = run_bass_kernel_spmd(
                nc, in_maps, list(range(N_CORES)), trace=TRACE
            )
            break
        except Exception as e:  # transient device wedge: retry
            last_err = e
            import time as _time

            _time.sleep(5)
    if res is None:
        raise last_err
    LAST_EXEC_NS = res.exec_time_ns
    globals()["_LAST_RES"] = res
    parts = [
        res.results[c]["out"].astype(np.float32) for c in range(N_CORES)
    ]
    parts2 = [
        res.results[c]["out2"].astype(np.float32) for c in range(N_CORES)
    ]
    out = np.empty((B, T, D), np.float32)
    for b in range(B):
        out[b] = parts[2 * b] + parts[2 * b + 1] + b_o[None, :]
        # exact bf16 recompute of the fp8-noisy first 128 rows
        out[b, 0:128] = parts2[2 * b] + parts2[2 * b + 1] + b_o[None, :]
    return out
